# revision 1
# baseline (speedup 1.0000x reference)
"""Trainium2 Bass kernel for 2-layer multi-head GAT (nn_GAT_38551626449703).

Strategy (8 NeuronCores, SPMD):
  - Nodes are partitioned uniformly: core k owns nodes [k*NPC, (k+1)*NPC).
  - Edges are sharded by OWNER OF src (softmax groups by src stay core-local).
  - Per core, edges are grouped into 128-node windows; each window has G groups
    of 128 edge-slots, split into an A-section (dst < 32767) and a B-section
    (dst >= 32767) so table rows fit int16 indices for dma_gather.
  - Gather tables (f32, dma_gather rows must be 256B-multiples):
      TW   [N+2, 320]: [Wh 4 heads (256) | s2 (4) | pad]  (by dst, rows n+1,
                        rows 0 / N+1 are sentinels with s2 = -1e30)
      TS1  [NPC, 64]:  [s1 (4) | pad]                     (by src, core-local)
      T2M  [N+2, 128]: [Wh2 (64) | s2o | pad]             (by dst, rows n+1)
      T2S1 [NPC, 64]:  [s1o | pad]                        (by src, core-local)
  - Pad slots gather sentinel rows (s2 = -1e30 -> exp(e) == 0 exactly).
  - Per window: dma_gather calls (<=768 idx each), e = lrelu(s1+s2), ex=exp(e)
    (no max-subtraction: |e| <= ~7 for this data, exp is safe in f32),
    R = [G*ex | ex], one-hot(src) matmul accumulates [u | denom] per node in
    PSUM across the window's G groups, then h' = u/denom, ELU.
  - Between layers only the compact T2M shard (3.2 MB) is AllGathered.
  - Outputs (rows for owned nodes) are concatenated on the host.
"""

import os
import sys

import numpy as np

sys.path.insert(0, "/opt/trn_rl_repo")

import concourse.bacc as bacc  # noqa: E402
import concourse.bass as bass  # noqa: E402
import concourse.tile as tile  # noqa: E402
from concourse import mybir  # noqa: E402
from concourse.masks import make_identity  # noqa: E402

F32 = mybir.dt.float32
I32 = mybir.dt.int32
I16 = mybir.dt.int16
AF = mybir.ActivationFunctionType
ALU = mybir.AluOpType

# Problem constants
N = 50000
E = 800000
F_IN = 128
HID = 64
HEADS = 4
OUT = 64
ALPHA = 0.2
CORES = 8

NEG = -1.0e30  # sentinel s2 -> exp(lrelu(s1+NEG)) == 0.0 in f32
HALF = 32767  # dst < HALF -> A section (table row dst+1 <= 32767)
QG = int(os.environ.get("GAT_QG", "6"))  # groups per dma_gather call
NSWQ = int(os.environ.get("GAT_NSWQ", "4"))  # SWDGE queues
F32R = bool(int(os.environ.get("GAT_F32R", "0")))  # PE fp32 fast mode


def _mm(ap):
    return ap.bitcast(mybir.dt.float32r) if F32R else ap


# Tile assigns the 8 DMASW completion-sem lanes round-robin over Pool-engine
# DMAs regardless of SWDGE queue, but a lane must stay on ONE queue (ucode
# constraint; violations -> corrupted sync / device crash). With NSWQ > 1 we
# partition the lanes: queue q owns lanes [q*8//NSWQ, (q+1)*8//NSWQ).
if NSWQ > 1:
    import concourse.bass_isa as _bass_isa
    import concourse.tile_sem_assignment as _tsa

    _orig_assign_tick = _tsa.TileClockTick._assign_tick

    def _lane_partitioned_assign_tick(self, inst):
        if (
            isinstance(inst, _tsa.DMAInst)
            and inst.engine == mybir.EngineType.Pool
            and not isinstance(inst, _bass_isa.UserSyncedRemoteDMADescs)
        ):
            qn = getattr(inst, "queue_num", 0) or 0
            per = getattr(self, "_q_lane_ctr", None)
            if per is None:
                per = self._q_lane_ctr = {}
            lanes = 8 // NSWQ
            c = per.get(qn, 0)
            per[qn] = c + 1
            self.next_sw_dma_idx = qn * lanes + (c % lanes)
        return _orig_assign_tick(self, inst)

    if _tsa.TileClockTick._assign_tick is not _lane_partitioned_assign_tick:
        _tsa.TileClockTick._assign_tick = _lane_partitioned_assign_tick
RW = 320  # TW row elements
R2W = 128  # T2M row elements


class Cfg:
    def __init__(self, n, cores, ka, kb):
        assert n % cores == 0
        self.n = n
        self.cores = cores
        self.npc = n // cores
        self.ka = ka  # A-section groups per window
        self.kb = kb  # B-section groups
        self.g = ka + kb
        self.nw = (self.npc + 127) // 128  # windows per core


def _calls(k0, k1):
    """Split groups [k0, k1) into dma_gather calls of <= QG groups."""
    out = []
    g = k0
    while g < k1:
        q = min(QG, k1 - g)
        out.append((g, q))
        g += q
    return out


def _elu(nc, pool, x_ap, tmp_shape):
    """elu(x) = relu(x) + exp(min(x, 0)) - 1"""
    p, c = tmp_shape
    t0 = pool.tile([p, c], F32, tag="elu_t0")
    t1 = pool.tile([p, c], F32, tag="elu_t1")
    nc.vector.tensor_scalar_min(t0[:], x_ap, 0.0)
    nc.scalar.activation(t0[:], t0[:], AF.Exp)
    nc.scalar.activation(t1[:], x_ap, AF.Relu)
    nc.vector.tensor_add(t1[:], t1[:], t0[:])
    nc.vector.tensor_scalar_add(t1[:], t1[:], -1.0)
    return t1


def build_nc(cfg: Cfg, dbg: bool = False, reps=None):
    """Build the SPMD Bass program (one program, runs on all cores)."""
    reps = reps or {}
    n, npc, G, NW = cfg.n, cfg.npc, cfg.g, cfg.nw
    KA, KB = cfg.ka, cfg.kb
    NT1 = (n + 127) // 128

    nc = bacc.Bacc(
        "TRN2", target_bir_lowering=False, debug=False, num_swdge_queues=NSWQ
    )

    # ---- external I/O ----
    xT_ext = nc.dram_tensor("xT", [F_IN, n], F32, kind="ExternalInput")
    xTo_ext = nc.dram_tensor("xTown", [F_IN, npc], F32, kind="ExternalInput")
    wh_ext = nc.dram_tensor("W_heads", [HEADS, F_IN, HID], F32, kind="ExternalInput")
    ah_ext = nc.dram_tensor("a_heads", [HEADS, 2 * HID], F32, kind="ExternalInput")
    wo_ext = nc.dram_tensor("W_out", [HEADS * HID, OUT], F32, kind="ExternalInput")
    ao_ext = nc.dram_tensor("a_out", [2 * OUT], F32, kind="ExternalInput")
    idx_d16 = nc.dram_tensor("idx_d16", [NW, 128, G * 8], I16, kind="ExternalInput")
    idx_s16 = nc.dram_tensor("idx_s16", [NW, 128, G * 8], I16, kind="ExternalInput")
    idx_srcl = nc.dram_tensor("idx_srcl", [NW, 128, G], I32, kind="ExternalInput")
    out_ext = nc.dram_tensor("out", [npc, OUT], F32, kind="ExternalOutput")

    # ---- internal DRAM ----
    tw = nc.dram_tensor("TW", [n + 2, RW], F32)
    ts1 = nc.dram_tensor("TS1", [npc, 64], F32)
    hcat = nc.dram_tensor("hcat", [npc, HEADS * HID], F32)
    t2msh = nc.dram_tensor("T2Msh", [npc, R2W], F32)
    t2s1 = nc.dram_tensor("T2S1", [npc, 64], F32)
    if cfg.cores > 1:
        t2m = nc.dram_tensor("T2M", [n + 2, R2W], F32, addr_space="Shared")
    else:
        t2m = nc.dram_tensor("T2M", [n + 2, R2W], F32)
    if dbg:
        dbg_tw = nc.dram_tensor("dbg_tw", [n + 2, RW], F32, kind="ExternalOutput")
        dbg_ts1 = nc.dram_tensor("dbg_ts1", [npc, 64], F32, kind="ExternalOutput")
        dbg_hcat = nc.dram_tensor(
            "dbg_hcat", [npc, HEADS * HID], F32, kind="ExternalOutput"
        )
        dbg_t2m = nc.dram_tensor("dbg_t2m", [n + 2, R2W], F32, kind="ExternalOutput")
        dbg_g = nc.dram_tensor("dbg_g", [128, G * RW], F32, kind="ExternalOutput")
        dbg_s1e = nc.dram_tensor("dbg_s1e", [128, G * 64], F32, kind="ExternalOutput")
        dbg_ex = nc.dram_tensor("dbg_ex", [128, G * HEADS], F32, kind="ExternalOutput")

    # dst-table call plan: A-section from row 0, B-section from row 32768.
    # Tile binds the 8 DMASW sem lanes to SWDGE DMAs round-robin in issue
    # order, and a lane must stay on one queue -- so pick the queue from a
    # global SWDGE-call counter as (c % 8) % NSWQ, which is constant per lane.
    d_calls = [(g0, q, 0) for g0, q in _calls(0, KA)] + [
        (g0, q, HALF + 1) for g0, q in _calls(KA, G)
    ]
    s_calls = _calls(0, G)
    swc = [0]

    def _q():
        qq = (swc[0] % 8) % NSWQ
        swc[0] += 1
        return qq

    with tile.TileContext(nc) as tc, tc.tile_pool(name="const", bufs=1) as cpool:
        with (
            tc.tile_pool(name="psW", bufs=1, space="PSUM") as psW,
            tc.tile_pool(name="psA", bufs=3, space="PSUM") as psA,
            tc.tile_pool(name="sbA", bufs=4) as sbA,
        ):
            # ======== constants ========
            ident = cpool.tile([128, 128], F32)
            make_identity(nc, ident[:])
            iota_i = sbA.tile([128, G * 128], I32, tag="iota_i")
            nc.gpsimd.iota(iota_i[:], [[0, G], [1, 128]], channel_multiplier=0)
            iota_t = cpool.tile([128, G * 128], F32)
            nc.vector.tensor_copy(iota_t[:], iota_i[:])

            # ======== Wext = [W_all(256) | c2(4) | c1(4)] on SBUF ========
            wext = cpool.tile([F_IN, HEADS * HID + 2 * HEADS], F32)
            nc.sync.dma_start(
                wext[:, 0 : HEADS * HID].rearrange("p (h o) -> p h o", h=HEADS),
                wh_ext[:].rearrange("h f o -> f h o"),
            )
            ps_c = psW.tile([128, 2 * HEADS], F32, tag="psc")
            for h in range(HEADS):
                wh_t = sbA.tile([F_IN, HID], F32, tag="wh_t")
                nc.sync.dma_start(wh_t[:], wh_ext[h])
                ps_w = psW.tile([HID, F_IN], F32, tag="psw")
                nc.tensor.transpose(ps_w[:], wh_t[:], ident[:])
                whT = sbA.tile([HID, F_IN], F32, tag="whT")
                nc.vector.tensor_copy(whT[:], ps_w[:])
                acol = sbA.tile([HID, 2], F32, tag="acol")
                nc.sync.dma_start(
                    acol[:], ah_ext[h : h + 1, :].rearrange("1 (t o) -> o t", t=2)
                )
                nc.tensor.matmul(
                    ps_c[:, 2 * h : 2 * h + 2], whT[:], acol[:], start=True, stop=True
                )
            nc.vector.tensor_copy(
                wext[:, HEADS * HID : HEADS * HID + HEADS], ps_c[:, 1 : 2 * HEADS : 2]
            )
            nc.vector.tensor_copy(
                wext[:, HEADS * HID + HEADS :], ps_c[:, 0 : 2 * HEADS : 2]
            )

            # ======== sentinel rows ========
            sent = sbA.tile([1, 260], F32, tag="sent")
            nc.vector.memset(sent[:], 0.0)
            nc.vector.memset(sent[:, HEADS * HID : HEADS * HID + HEADS], NEG)
            nc.sync.dma_start(tw[0:1, 0:260], sent[:])
            nc.sync.dma_start(tw[n + 1 : n + 2, 0:260], sent[:])
            sent3 = sbA.tile([1, R2W], F32, tag="sent3")
            nc.vector.memset(sent3[:], 0.0)
            nc.vector.memset(sent3[:, OUT : OUT + 1], NEG)
            nc.sync.dma_start(t2m[0:1, :], sent3[:])
            nc.sync.dma_start(t2m[n + 1 : n + 2, :], sent3[:])

            # ======== phase A: build TW (all nodes) + TS1 (own nodes) ======
            for _ra in range(reps.get("A", 1)):
                for t in range(NT1):
                    n0 = 128 * t
                    rows = min(128, n - n0)
                    xT_t = sbA.tile([F_IN, 128], F32, tag="xT_t")
                    if rows < 128:
                        nc.vector.memset(xT_t[:], 0.0)
                    nc.sync.dma_start(xT_t[:, :rows], xT_ext[:, n0 : n0 + rows])
                    ps_o = psA.tile([128, HEADS * HID + 2 * HEADS], F32, tag="psA_o")
                    nc.tensor.matmul(
                        ps_o[:], _mm(xT_t[:]), _mm(wext[:]), start=True, stop=True
                    )
                    ot = sbA.tile([128, 260], F32, tag="otA")
                    nc.vector.tensor_copy(ot[:], ps_o[:, 0:260])
                    nc.sync.dma_start(tw[1 + n0 : 1 + n0 + rows, 0:260], ot[:rows, :])
                # TS1 (own nodes): s1 = x_own @ c1
                for t in range(NW):
                    n0 = 128 * t
                    rows = min(128, npc - n0)
                    xo_t = sbA.tile([F_IN, 128], F32, tag="xo_t")
                    if rows < 128:
                        nc.vector.memset(xo_t[:], 0.0)
                    nc.sync.dma_start(xo_t[:, :rows], xTo_ext[:, n0 : n0 + rows])
                    ps_s = psA.tile([128, HEADS], F32, tag="psA_s")
                    nc.tensor.matmul(
                        ps_s[:],
                        xo_t[:],
                        wext[:, HEADS * HID + HEADS :],
                        start=True,
                        stop=True,
                    )
                    os_t = sbA.tile([128, 64], F32, tag="osA")
                    nc.vector.memset(os_t[:, HEADS:], 0.0)
                    nc.vector.tensor_copy(os_t[:, 0:HEADS], ps_s[:])
                    nc.sync.dma_start(ts1[n0 : n0 + rows, :], os_t[:rows, :])

        # ======== phase B(g): gathers only (benchmark variant) ========
        if reps.get("Bg"):
            with tc.tile_pool(name="sbBg", bufs=2) as sbG:
                for _rg in range(reps["Bg"]):
                    for w in range(NW):
                        i16d = sbG.tile([128, G * 8], I16, tag="i16d")
                        nc.sync.dma_start(i16d[:], idx_d16[w])
                        i16s = sbG.tile([128, G * 8], I16, tag="i16s")
                        nc.sync.dma_start(i16s[:], idx_s16[w])
                        g_t = sbG.tile([128, G * RW], F32, tag="g_t")
                        for g0, q, base in d_calls:
                            nc.gpsimd.dma_gather(
                                g_t[:, g0 * RW : (g0 + q) * RW].rearrange(
                                    "p (k e) -> p k e", e=RW
                                ),
                                tw[base:, :] if base else tw[:],
                                i16d[:, g0 * 8 : (g0 + q) * 8],
                                q * 128,
                                q * 128,
                                RW,
                                queue_num=_q(),
                            )
                        s1e = sbG.tile([128, G * 64], F32, tag="s1e")
                        for g0, q in s_calls:
                            nc.gpsimd.dma_gather(
                                s1e[:, g0 * 64 : (g0 + q) * 64].rearrange(
                                    "p (k e) -> p k e", e=64
                                ),
                                ts1[:],
                                i16s[:, g0 * 8 : (g0 + q) * 8],
                                q * 128,
                                q * 128,
                                64,
                                queue_num=_q(),
                            )
                        # consume so nothing is dead
                        acc = sbG.tile([128, 4], F32, tag="accg")
                        nc.vector.tensor_copy(acc[:], g_t[:, 0:4])

        # ======== phase B: layer-1 edge processing ========
        with (
            tc.tile_pool(name="psB", bufs=6, space="PSUM") as psB,
            tc.tile_pool(name="sbB", bufs=3) as sbB,
        ):
            for _rb in range(reps.get("B", 1)):
                for w in range(NW):
                    wn = min(128, npc - 128 * w)
                    i16d = sbB.tile([128, G * 8], I16, tag="i16d")
                    nc.sync.dma_start(i16d[:], idx_d16[w])
                    i16s = sbB.tile([128, G * 8], I16, tag="i16s")
                    nc.sync.dma_start(i16s[:], idx_s16[w])
                    srcl_i = sbB.tile([128, G], I32, tag="srcl_i")
                    nc.sync.dma_start(srcl_i[:], idx_srcl[w])
                    srcl_f = sbB.tile([128, G], F32, tag="srcl_f")
                    nc.vector.tensor_copy(srcl_f[:], srcl_i[:])

                    g_t = sbB.tile([128, G * RW], F32, tag="g_t")
                    for g0, q, base in d_calls:
                        nc.gpsimd.dma_gather(
                            g_t[:, g0 * RW : (g0 + q) * RW].rearrange(
                                "p (k e) -> p k e", e=RW
                            ),
                            tw[base:, :] if base else tw[:],
                            i16d[:, g0 * 8 : (g0 + q) * 8],
                            q * 128,
                            q * 128,
                            RW,
                            queue_num=_q(),
                        )
                    s1e = sbB.tile([128, G * 64], F32, tag="s1e")
                    for g0, q in s_calls:
                        nc.gpsimd.dma_gather(
                            s1e[:, g0 * 64 : (g0 + q) * 64].rearrange(
                                "p (k e) -> p k e", e=64
                            ),
                            ts1[:],
                            i16s[:, g0 * 8 : (g0 + q) * 8],
                            q * 128,
                            q * 128,
                            64,
                            queue_num=_q(),
                        )

                    # e = lrelu(s1 + s2) ; ex = exp(e)
                    g3 = g_t[:].rearrange("p (g c) -> p g c", c=RW)
                    s13 = s1e[:].rearrange("p (g c) -> p g c", c=64)
                    ex_t = sbB.tile([128, G * HEADS], F32, tag="ex_t")
                    nc.vector.tensor_add(
                        ex_t[:].rearrange("p (g h) -> p g h", h=HEADS),
                        s13[:, :, 0:HEADS],
                        g3[:, :, HEADS * HID : HEADS * HID + HEADS],
                    )
                    lr_t = sbB.tile([128, G * HEADS], F32, tag="lr_t")
                    nc.vector.tensor_scalar_mul(lr_t[:], ex_t[:], ALPHA)
                    nc.vector.tensor_tensor(ex_t[:], ex_t[:], lr_t[:], op=ALU.max)
                    nc.scalar.activation(ex_t[:], ex_t[:], AF.Exp)

                    # onehot[e, 128*g + j] = (srcl[e,g] == j)
                    oh = sbB.tile([128, G * 128], F32, tag="oh")
                    nc.vector.tensor_tensor(
                        out=oh[:].rearrange("p (g j) -> p g j", j=128),
                        in0=srcl_f[:].unsqueeze(2).to_broadcast([128, G, 128]),
                        in1=iota_t[:].rearrange("p (g j) -> p g j", j=128),
                        op=ALU.is_equal,
                    )

                    # R = [G*ex | ex] built in place in g_t (s2 slot -> ex)
                    ex3 = ex_t[:].rearrange("p (g h) -> p g h", h=HEADS)
                    nc.vector.tensor_tensor(
                        out=g3[:, :, 0 : HEADS * HID].rearrange(
                            "p g (h o) -> p g h o", h=HEADS
                        ),
                        in0=g3[:, :, 0 : HEADS * HID].rearrange(
                            "p g (h o) -> p g h o", h=HEADS
                        ),
                        in1=ex3.unsqueeze(3).to_broadcast([128, G, HEADS, HID]),
                        op=ALU.mult,
                    )
                    nc.vector.tensor_copy(
                        g3[:, :, HEADS * HID : HEADS * HID + HEADS], ex3
                    )

                    ps_u = psB.tile([128, 260], F32, tag="ps_u")
                    for g in range(G):
                        nc.tensor.matmul(
                            ps_u[:],
                            _mm(oh[:, g * 128 : (g + 1) * 128]),
                            _mm(g_t[:, g * RW : g * RW + 260]),
                            start=(g == 0),
                            stop=(g == G - 1),
                        )

                    u_t = sbB.tile([128, 260], F32, tag="u_t")
                    nc.vector.tensor_copy(u_t[:], ps_u[:])
                    r4 = sbB.tile([128, HEADS], F32, tag="r4")
                    nc.vector.tensor_scalar_add(
                        r4[:], u_t[:, HEADS * HID : 260], 1e-30
                    )
                    nc.vector.reciprocal(r4[:], r4[:])
                    hp = sbB.tile([128, HEADS * HID], F32, tag="hp")
                    nc.vector.tensor_tensor(
                        out=hp[:].rearrange("p (h o) -> p h o", h=HEADS),
                        in0=u_t[:, 0 : HEADS * HID].rearrange(
                            "p (h o) -> p h o", h=HEADS
                        ),
                        in1=r4[:].unsqueeze(2).to_broadcast([128, HEADS, HID]),
                        op=ALU.mult,
                    )
                    he = _elu(nc, sbB, hp[:], [128, HEADS * HID])
                    nc.sync.dma_start(hcat[128 * w : 128 * w + wn, :], he[:wn, :])
                    if dbg and w == 0:
                        nc.sync.dma_start(dbg_g[:], g_t[:])
                        nc.sync.dma_start(dbg_s1e[:], s1e[:])
                        nc.sync.dma_start(dbg_ex[:], ex_t[:])

        # ======== phase C: build own T2M / T2S1 shards ========
        with (
            tc.tile_pool(name="psC", bufs=2, space="PSUM") as psC,
            tc.tile_pool(name="sbC", bufs=3) as sbC,
            tc.tile_pool(name="cc", bufs=1) as ccpool,
        ):
            # W2ext chunks [128, 66] x2 : [W_out | c2o | c1o]
            w2e = []
            for c in range(2):
                w2c = ccpool.tile([128, OUT + 2], F32, tag=f"w2e{c}")
                nc.sync.dma_start(w2c[:, 0:OUT], wo_ext[128 * c : 128 * (c + 1), :])
                wo_t = sbC.tile([128, OUT], F32, tag="wo_t")
                nc.sync.dma_start(wo_t[:], wo_ext[128 * c : 128 * (c + 1), :])
                ps_w = psC.tile([OUT, 128], F32, tag="psw2")
                nc.tensor.transpose(ps_w[:], wo_t[:], ident[:])
                woT = sbC.tile([OUT, 128], F32, tag="woT")
                nc.vector.tensor_copy(woT[:], ps_w[:])
                aoc = sbC.tile([OUT, 2], F32, tag="aoc")
                nc.sync.dma_start(
                    aoc[:], ao_ext[:].unsqueeze(0).rearrange("1 (t o) -> o t", t=2)
                )
                ps_c2 = psC.tile([128, 2], F32, tag="psc2")
                nc.tensor.matmul(ps_c2[:], woT[:], aoc[:], start=True, stop=True)
                nc.vector.tensor_copy(w2c[:, OUT : OUT + 1], ps_c2[:, 1:2])
                nc.vector.tensor_copy(w2c[:, OUT + 1 : OUT + 2], ps_c2[:, 0:1])
                w2e.append(w2c)

            for _rc in range(reps.get("C", 1)):
                for t in range(NW):
                    n0 = 128 * t
                    rows = min(128, npc - n0)
                    ht = sbC.tile([128, HEADS * HID], F32, tag="ht")
                    if rows < 128:
                        nc.vector.memset(ht[:], 0.0)
                    nc.sync.dma_start(ht[:rows, :], hcat[n0 : n0 + rows, :])
                    ps_o = psC.tile([128, OUT + 2], F32, tag="psC_o")
                    for c in range(2):
                        ps_t = psC.tile([128, 128], F32, tag="psC_t")
                        nc.tensor.transpose(
                            ps_t[:], ht[:, 128 * c : 128 * (c + 1)], ident[:]
                        )
                        hT = sbC.tile([128, 128], F32, tag="hT")
                        nc.vector.tensor_copy(hT[:], ps_t[:])
                        nc.tensor.matmul(
                            ps_o[:], hT[:], w2e[c][:], start=(c == 0), stop=(c == 1)
                        )
                    ot = sbC.tile([128, R2W], F32, tag="otC")
                    nc.vector.memset(ot[:, OUT + 1 :], 0.0)
                    nc.vector.tensor_copy(ot[:, 0 : OUT + 1], ps_o[:, 0 : OUT + 1])
                    nc.sync.dma_start(t2msh[n0 : n0 + rows, :], ot[:rows, :])
                    os_t = sbC.tile([128, 64], F32, tag="osC")
                    nc.vector.memset(os_t[:, 1:], 0.0)
                    nc.vector.tensor_copy(os_t[:, 0:1], ps_o[:, OUT + 1 : OUT + 2])
                    nc.sync.dma_start(t2s1[n0 : n0 + rows, :], os_t[:rows, :])

        # ======== phase D: allgather T2M ========
        if cfg.cores > 1:
            nc.gpsimd.collective_compute(
                "AllGather",
                ALU.bypass,
                replica_groups=[list(range(cfg.cores))],
                ins=[t2msh[:]],
                outs=[t2m[1 : n + 1, :]],
            )
        else:
            nc.sync.dma_start(t2m[1 : n + 1, :], t2msh[:])

        # ======== phase E: layer-2 edge processing ========
        with (
            tc.tile_pool(name="psE", bufs=6, space="PSUM") as psE,
            tc.tile_pool(name="sbE", bufs=5) as sbE,
        ):
            for _re in range(reps.get("E", 1)):
                for w in range(NW):
                    wn = min(128, npc - 128 * w)
                    i16d = sbE.tile([128, G * 8], I16, tag="i16d")
                    nc.sync.dma_start(i16d[:], idx_d16[w])
                    i16s = sbE.tile([128, G * 8], I16, tag="i16s")
                    nc.sync.dma_start(i16s[:], idx_s16[w])
                    srcl_i = sbE.tile([128, G], I32, tag="srcl_i")
                    nc.sync.dma_start(srcl_i[:], idx_srcl[w])
                    srcl_f = sbE.tile([128, G], F32, tag="srcl_f")
                    nc.vector.tensor_copy(srcl_f[:], srcl_i[:])

                    g_t = sbE.tile([128, G * R2W], F32, tag="g_t2")
                    for g0, q, base in d_calls:
                        nc.gpsimd.dma_gather(
                            g_t[:, g0 * R2W : (g0 + q) * R2W].rearrange(
                                "p (k e) -> p k e", e=R2W
                            ),
                            t2m[base:, :] if base else t2m[:],
                            i16d[:, g0 * 8 : (g0 + q) * 8],
                            q * 128,
                            q * 128,
                            R2W,
                            queue_num=_q(),
                        )
                    s1e = sbE.tile([128, G * 64], F32, tag="s1e2")
                    for g0, q in s_calls:
                        nc.gpsimd.dma_gather(
                            s1e[:, g0 * 64 : (g0 + q) * 64].rearrange(
                                "p (k e) -> p k e", e=64
                            ),
                            t2s1[:],
                            i16s[:, g0 * 8 : (g0 + q) * 8],
                            q * 128,
                            q * 128,
                            64,
                            queue_num=_q(),
                        )

                    g3 = g_t[:].rearrange("p (g c) -> p g c", c=R2W)
                    s13 = s1e[:].rearrange("p (g c) -> p g c", c=64)
                    ex_t = sbE.tile([128, G], F32, tag="ex_t2")
                    nc.vector.tensor_add(
                        ex_t[:].unsqueeze(2), s13[:, :, 0:1], g3[:, :, OUT : OUT + 1]
                    )
                    lr_t = sbE.tile([128, G], F32, tag="lr_t2")
                    nc.vector.tensor_scalar_mul(lr_t[:], ex_t[:], ALPHA)
                    nc.vector.tensor_tensor(ex_t[:], ex_t[:], lr_t[:], op=ALU.max)
                    nc.scalar.activation(ex_t[:], ex_t[:], AF.Exp)

                    oh = sbE.tile([128, G * 128], F32, tag="oh")
                    nc.vector.tensor_tensor(
                        out=oh[:].rearrange("p (g j) -> p g j", j=128),
                        in0=srcl_f[:].unsqueeze(2).to_broadcast([128, G, 128]),
                        in1=iota_t[:].rearrange("p (g j) -> p g j", j=128),
                        op=ALU.is_equal,
                    )

                    nc.vector.tensor_tensor(
                        out=g3[:, :, 0:OUT],
                        in0=g3[:, :, 0:OUT],
                        in1=ex_t[:].unsqueeze(2).to_broadcast([128, G, OUT]),
                        op=ALU.mult,
                    )
                    nc.vector.tensor_copy(
                        g3[:, :, OUT : OUT + 1], ex_t[:].unsqueeze(2)
                    )

                    ps_u = psE.tile([128, OUT + 1], F32, tag="ps_u2")
                    for g in range(G):
                        nc.tensor.matmul(
                            ps_u[:],
                            _mm(oh[:, g * 128 : (g + 1) * 128]),
                            _mm(g_t[:, g * R2W : g * R2W + OUT + 1]),
                            start=(g == 0),
                            stop=(g == G - 1),
                        )

                    u_t = sbE.tile([128, OUT + 1], F32, tag="u_t2")
                    nc.vector.tensor_copy(u_t[:], ps_u[:])
                    r4 = sbE.tile([128, 1], F32, tag="r42")
                    nc.vector.tensor_scalar_add(r4[:], u_t[:, OUT : OUT + 1], 1e-30)
                    nc.vector.reciprocal(r4[:], r4[:])
                    op_t = sbE.tile([128, OUT], F32, tag="op_t")
                    nc.vector.tensor_tensor(
                        out=op_t[:],
                        in0=u_t[:, 0:OUT],
                        in1=r4[:].to_broadcast([128, OUT]),
                        op=ALU.mult,
                    )
                    oe = _elu(nc, sbE, op_t[:], [128, OUT])
                    nc.sync.dma_start(out_ext[128 * w : 128 * w + wn, :], oe[:wn, :])

        if dbg:
            nc.sync.dma_start(dbg_tw[:], tw[:])
            nc.sync.dma_start(dbg_ts1[:], ts1[:])
            nc.sync.dma_start(dbg_hcat[:], hcat[:])
            nc.sync.dma_start(dbg_t2m[:], t2m[:])

    nc.compile()
    return nc


# ---------------------------------------------------------------------------
# Host-side preparation and execution
# ---------------------------------------------------------------------------


def _pack16_slots(slot_vals, nw, g):
    """slot_vals [NW, G*128] in slot order j -> [NW, 128, G*8] int16 layout:
    idx j at [16*r + j%16, j//16], replicated for r in 0..7."""
    w = slot_vals.reshape(nw, g * 8, 16)  # [NW, j//16, j%16]
    w = np.swapaxes(w, 1, 2)  # [NW, 16, G*8]
    return np.ascontiguousarray(np.tile(w, (1, 8, 1)).astype(np.int16))


def section_sizes(n, cores, edges):
    """Max A/B-section group counts over all (core, window)."""
    npc = n // cores
    nw = (npc + 127) // 128
    src = np.asarray(edges[0], dtype=np.int64)
    dst = np.asarray(edges[1], dtype=np.int64)
    order = np.argsort(src, kind="stable")
    ssrc, sdst = src[order], dst[order]
    ka = kb = 0
    for k in range(cores):
        for w in range(nw):
            lo = k * npc + 128 * w
            hi = min(lo + 128, (k + 1) * npc)
            s0, s1 = np.searchsorted(ssrc, [lo, hi])
            d = sdst[s0:s1]
            ca = int((d < HALF).sum())
            cb = int(len(d) - ca)
            ka = max(ka, max(1, -(-ca // 128)))
            kb = max(kb, -(-cb // 128))
    if n + 1 > HALF:
        kb = max(kb, 1)
    return ka, kb


def prepare_inputs(cfg: Cfg, x, edges, W_heads, a_heads, W_out, a_out):
    """Build per-core input maps. Pure layout/index manipulation."""
    n, cores, npc, G, NW = cfg.n, cfg.cores, cfg.npc, cfg.g, cfg.nw
    KA = cfg.ka
    src = np.asarray(edges[0], dtype=np.int64)
    dst = np.asarray(edges[1], dtype=np.int64)
    order = np.argsort(src, kind="stable")
    ssrc = src[order]
    sdst = dst[order]

    xT = np.ascontiguousarray(np.asarray(x, np.float32).T)

    common = dict(
        xT=xT,
        W_heads=np.asarray(W_heads, np.float32),
        a_heads=np.asarray(a_heads, np.float32),
        W_out=np.asarray(W_out, np.float32),
        a_out=np.asarray(a_out, np.float32),
    )

    in_maps = []
    for k in range(cores):
        vd = np.zeros((NW, G * 128), dtype=np.int64)
        vs = np.zeros((NW, G * 128), dtype=np.int64)
        vl = np.zeros((NW, 128, G), dtype=np.int32)
        for w in range(NW):
            lo = k * npc + 128 * w
            hi = min(lo + 128, (k + 1) * npc)
            s0, s1 = np.searchsorted(ssrc, [lo, hi])
            d, s = sdst[s0:s1], ssrc[s0:s1]
            selA = d < HALF
            dA, sA = d[selA], s[selA]
            dB, sB = d[~selA], s[~selA]
            assert len(dA) <= 128 * KA and len(dB) <= 128 * (G - KA)
            # dst rows (A: row dst+1, pads row 0; B: local row, pads sentinel)
            rowA = np.zeros(128 * KA, dtype=np.int64)
            rowA[: len(dA)] = dA + 1
            rowB = np.full(128 * (G - KA), n + 1 - (HALF + 1), dtype=np.int64)
            rowB[: len(dB)] = dB + 1 - (HALF + 1)
            vd[w] = np.concatenate([rowA, rowB])
            # src rows (core-local; pads -> 0, harmless)
            sl = np.zeros(128 * KA, dtype=np.int64)
            sl[: len(sA)] = sA - k * npc
            sl2 = np.zeros(128 * (G - KA), dtype=np.int64)
            sl2[: len(sB)] = sB - k * npc
            vs[w] = np.concatenate([sl, sl2])
            # window-local src for the one-hot (pads -> 0)
            wl = np.zeros(128 * KA, dtype=np.int32)
            wl[: len(sA)] = (sA - lo).astype(np.int32)
            wl2 = np.zeros(128 * (G - KA), dtype=np.int32)
            wl2[: len(sB)] = (sB - lo).astype(np.int32)
            allw = np.concatenate([wl, wl2])
            vl[w] = allw.reshape(G, 128).T  # slot j = g*128 + p
        in_maps.append(
            dict(
                common,
                xTown=np.ascontiguousarray(xT[:, k * npc : (k + 1) * npc]),
                idx_d16=_pack16_slots(vd, NW, G),
                idx_s16=_pack16_slots(vs, NW, G),
                idx_srcl=vl,
            )
        )
    return in_maps


_NC_CACHE = {}


def get_nc(cfg: Cfg):
    key = (cfg.n, cfg.cores, cfg.ka, cfg.kb)
    if key not in _NC_CACHE:
        _NC_CACHE[key] = build_nc(cfg)
    return _NC_CACHE[key]


def make_cfg(n, cores, edges):
    ka, kb = section_sizes(n, cores, edges)
    return Cfg(n, cores, ka, kb)


def run(inputs, trace=False, **spmd_kwargs):
    from concourse.bass_utils import run_bass_kernel_spmd

    x = np.asarray(inputs["x"], np.float32)
    edges = np.asarray(inputs["edges"])
    cfg = make_cfg(N, CORES, edges)
    nc = get_nc(cfg)
    in_maps = prepare_inputs(
        cfg,
        x,
        edges,
        inputs["W_heads"],
        inputs["a_heads"],
        inputs["W_out"],
        inputs["a_out"],
    )
    res = run_bass_kernel_spmd(
        nc, in_maps, core_ids=list(range(CORES)), trace=trace, **spmd_kwargs
    )
    out = np.concatenate([r["out"] for r in res.results], axis=0)
    return out, res


def kernel(**inputs):
    return run(inputs)[0]



# revision 5
# speedup vs baseline: 985.0573x; 985.0573x over previous
"""Trainium2 Bass kernel v2 for 2-layer multi-head GAT (nn_GAT_38551626449703).

Design (8 NeuronCores, SPMD, one shared program):
  - Core k owns nodes [k*NPC, (k+1)*NPC).  Within a core, owned nodes are
    PERMUTED by out-degree (descending) and windowed 128 at a time; the
    shared per-window group count G_w = max over cores of the window's max
    degree (3-4% slot padding).  Host un-permutes the output rows.
  - Edge slots: window w, partition p, group g holds the g-th edge of the
    window's p-th node, so PARTITION p == SRC NODE p.  No one-hot matrix,
    no src gather: per-node src factors broadcast along the free dim.
  - exp/LeakyReLU factorization (exact, since exp is monotonic):
        ex = exp(lrelu(s1+s2)) = max(p1[src]*q2[dst], p1'[src]*q2'[dst])
    with p1=exp(s1), p1'=exp(.2*s1) per src node (SBUF-resident) and
    q2=exp(s2), q2'=exp(.2*s2) stored per dst node in the gather tables.
  - Gather tables (row dtype bf16; the f32 q-pairs live in the row pad):
      TW  [N2+1, 384]: [Wh (o,h)-order 256 | q2 f32x4 | q2' f32x4 | pad]
                        768B rows; row N2 is an all-zero sentinel.
      T2M [N+1, 128]:  [Wh2 64 | q2o f32 | q2o' f32 | pad] 256B rows;
                        rows are permuted-block (owner*NPC + rank);
                        row N is the sentinel.
    One dma_gather per window per layer: int16 indices with table base at
    row RB=32768 cover all rows via SIGNED offsets (verified on HW).
    Pad slots gather the sentinel row (q2=0 -> ex==0 exactly).
  - Accumulation: per group g one matmul with a STATIONARY bf16 identity,
    rhs = [Wh*ex | ex] -> PSUM f32 [128, 260] accumulates numerator+denom.
  - Wh columns are stored in (o,h) order so the ex broadcast multiplies
    keep the last AP dim packed (DVE 2x mode).
  - Between layers only the compact T2M shard (1.6 MB bf16) is AllGathered.
"""

import os
import sys

import numpy as np

sys.path.insert(0, "/opt/trn_rl_repo")

import concourse.bacc as bacc  # noqa: E402
import concourse.tile as tile  # noqa: E402
from concourse import mybir  # noqa: E402
from concourse.masks import make_identity  # noqa: E402

F32 = mybir.dt.float32
BF16 = mybir.dt.bfloat16
I16 = mybir.dt.int16
AF = mybir.ActivationFunctionType
ALU = mybir.AluOpType

# Problem constants
N = 50000
E = 800000
F_IN = 128
HID = 64
HEADS = 4
OUT = 64
ALPHA = 0.2
CORES = 8
NPC = N // CORES  # 6250
NW = (NPC + 127) // 128  # 49

RB = 32768  # gather table base row (int16 signed-offset trick)
GSPLIT = 6  # max groups per dma_gather call (descriptor-ring capacity)
RW1 = 384  # TW row, bf16 elems (768B)
RW2 = 128  # T2M row, bf16 elems (256B)
ACHUNK = 2048  # phase-A nodes per chunk
N2 = ((N + ACHUNK - 1) // ACHUNK) * ACHUNK  # 51200; rows N..N2-1 zero-x junk
NT1 = N2 // ACHUNK  # 25
SENT1 = N2  # TW sentinel row
SENT2 = N  # T2M sentinel row

NSWQ = int(os.environ.get("GAT_NSWQ", "4"))  # SWDGE queues

# Tile assigns the 8 DMASW completion-sem lanes round-robin over Pool-engine
# DMAs regardless of SWDGE queue, but a lane must stay on ONE queue (ucode
# constraint).  With NSWQ > 1 we partition the lanes per queue.
if NSWQ > 1:
    import concourse.bass_isa as _bass_isa
    import concourse.tile_sem_assignment as _tsa

    _orig_assign_tick = _tsa.TileClockTick._assign_tick

    def _lane_partitioned_assign_tick(self, inst):
        if (
            isinstance(inst, _tsa.DMAInst)
            and inst.engine == mybir.EngineType.Pool
            and not isinstance(inst, _bass_isa.UserSyncedRemoteDMADescs)
        ):
            qn = getattr(inst, "queue_num", 0) or 0
            per = getattr(self, "_q_lane_ctr", None)
            if per is None:
                per = self._q_lane_ctr = {}
            lanes = 8 // NSWQ
            c = per.get(qn, 0)
            per[qn] = c + 1
            self.next_sw_dma_idx = qn * lanes + (c % lanes)
        return _orig_assign_tick(self, inst)

    if _tsa.TileClockTick._assign_tick is not _lane_partitioned_assign_tick:
        _tsa.TileClockTick._assign_tick = _lane_partitioned_assign_tick


class Cfg:
    def __init__(self, gs):
        self.gs = tuple(int(g) for g in gs)  # per-window group counts
        assert len(self.gs) == NW
        self.sg = sum(self.gs)
        self.goff = np.concatenate([[0], np.cumsum(self.gs)]).astype(int)
        self.gmax = max(self.gs)
        # per-window gather calls: (s0, sq, idx col offset).  Each call's
        # index list gets 16 appended sentinel slots (idx >= 0) because the
        # ucode SKIPS a trailing run of negative indices (HW-probed); the
        # junk slots land in group s0+sq, partitions 0..15.
        self.calls = []
        off = 0
        for g in self.gs:
            wcalls = []
            for s0 in range(0, g, GSPLIT):
                sq = min(GSPLIT, g - s0)
                wcalls.append((s0, sq, off))
                off += sq * 8 + 1
            self.calls.append(wcalls)
        self.icols = off


def _elu_bf(nc, pool, x_ap, cols, out_ap, tag, dt=BF16):
    """out = elu(x) = relu(x) + exp(x - relu(x)) - 1.  relu/exp on ACT."""
    rl = pool.tile([128, cols], dt, tag=f"{tag}_rl")
    nc.scalar.activation(rl[:], x_ap, AF.Relu)
    t = pool.tile([128, cols], dt, tag=f"{tag}_t")
    nc.vector.tensor_tensor(t[:], x_ap, rl[:], op=ALU.subtract)
    nc.scalar.activation(t[:], t[:], AF.Exp)
    # out = (rl + (-1)) + t
    nc.vector.scalar_tensor_tensor(
        out=out_ap, in0=rl[:], scalar=-1.0, in1=t[:], op0=ALU.add, op1=ALU.add
    )


def build_nc(cfg: Cfg, reps=None, sim_collective: bool = False):
    reps = reps or {}
    phases = os.environ.get("GAT_PHASES", "ABCDE")
    gs, goff = cfg.gs, cfg.goff
    SG8 = 8 * cfg.sg
    GMAX = cfg.gmax

    # One dma_gather call's descriptors must fit the SWDGE ring (default
    # dynamic_dma_scratch 16384B = 1024 descriptors); GSPLIT=6 groups
    # (768+16 descriptors) is the HW-proven safe call size.
    nc = bacc.Bacc(
        "TRN2",
        target_bir_lowering=False,
        debug=False,
        num_swdge_queues=NSWQ,
    )

    # ---- external I/O ----
    xT_ext = nc.dram_tensor("xT", [F_IN, N2], BF16, kind="ExternalInput")
    xTo_ext = nc.dram_tensor("xTown", [F_IN, NW * 128], BF16, kind="ExternalInput")
    wext_ext = nc.dram_tensor("wext", [F_IN, 272], BF16, kind="ExternalInput")
    w2ext_ext = nc.dram_tensor("w2ext", [2, 128, 68], BF16, kind="ExternalInput")
    idx1_ext = nc.dram_tensor("idx1", [128, cfg.icols], I16, kind="ExternalInput")
    idx2_ext = nc.dram_tensor("idx2", [128, cfg.icols], I16, kind="ExternalInput")
    out_ext = nc.dram_tensor("out", [NPC, OUT], F32, kind="ExternalOutput")

    # ---- internal DRAM ----
    tw = nc.dram_tensor("TW", [N2 + 1, RW1], BF16)
    t2msh = nc.dram_tensor("T2Msh", [NPC, RW2], BF16)
    if CORES > 1 and not sim_collective:
        t2m = nc.dram_tensor("T2M", [N + 1, RW2], BF16, addr_space="Shared")
    else:
        t2m = nc.dram_tensor("T2M", [N + 1, RW2], BF16)

    def q_of(w):
        return w % NSWQ

    with tile.TileContext(nc) as tc, tc.tile_pool(name="const", bufs=1) as cpool:
        # ======== persistent SBUF ========
        ident = cpool.tile([128, 128], F32)
        make_identity(nc, ident[:])
        identb = cpool.tile([128, 128], BF16)
        nc.vector.tensor_copy(identb[:], ident[:])
        wextsb = cpool.tile([F_IN, 272], BF16)
        nc.sync.dma_start(wextsb[:], wext_ext[:])
        w2sb = cpool.tile([128, 2 * 68], BF16)
        nc.sync.dma_start(w2sb[:, 0:68], w2ext_ext[0])
        nc.sync.dma_start(w2sb[:, 68:136], w2ext_ext[1])
        i16_1 = cpool.tile([128, cfg.icols], I16)
        nc.sync.dma_start(i16_1[:], idx1_ext[:])
        i16_2 = cpool.tile([128, cfg.icols], I16)
        nc.sync.dma_start(i16_2[:], idx2_ext[:])
        p1sb = cpool.tile([128, NW * 8], F32)
        p1osb = cpool.tile([128, NW * 2], F32)
        hcat = cpool.tile([128, NW * 256], BF16)
        out_all = cpool.tile([128, NW * OUT], F32)
        xo_all = cpool.tile([128, NW * 128], BF16)
        nc.sync.dma_start(xo_all[:], xTo_ext[:])

        # sentinel rows (all zeros)
        z1 = cpool.tile([1, RW1], BF16)
        nc.vector.memset(z1[:], 0.0)
        nc.sync.dma_start(tw[SENT1 : SENT1 + 1, :], z1[:])
        z2 = cpool.tile([1, RW2], BF16)
        nc.vector.memset(z2[:], 0.0)
        nc.sync.dma_start(t2m[SENT2 : SENT2 + 1, :], z2[:])

        # ======== phase A: build TW (all nodes) + p1/p1' (own nodes) ======
        with (
            tc.tile_pool(name="psA", bufs=2, space="PSUM") as psA,
            tc.tile_pool(name="sbA", bufs=3) as sbA,
        ):
            for _ra in range(reps.get("A", 1) if "A" in phases else 0):
                for t in range(NT1):
                    c0 = ACHUNK * t
                    xt = sbA.tile([128, ACHUNK], BF16, tag="xt")
                    nc.sync.dma_start(xt[:], xT_ext[:, c0 : c0 + ACHUNK])
                    wt = sbA.tile([128, (ACHUNK // 128) * 256], BF16, tag="wt")
                    ps_q = psA.tile([128, (ACHUNK // 128) * 8], F32, tag="ps_q")
                    for par in range(ACHUNK // 256):
                        ps_wh = psA.tile([128, 512], F32, tag="ps_wh")
                        for h in range(2):
                            q = 2 * par + h
                            nc.tensor.matmul(
                                ps_wh[:, 256 * h : 256 * (h + 1)],
                                xt[:, 128 * q : 128 * (q + 1)],
                                wextsb[:, 0:256],
                                start=True,
                                stop=True,
                            )
                            nc.tensor.matmul(
                                ps_q[:, 8 * q : 8 * (q + 1)],
                                xt[:, 128 * q : 128 * (q + 1)],
                                wextsb[:, 256:264],
                                start=True,
                                stop=True,
                            )
                        # convert Wh pair to bf16 (alternate ACT / DVE)
                        dst = wt[:, 512 * par : 512 * (par + 1)]
                        if par % 2 == 0:
                            nc.scalar.copy(dst, ps_wh[:])
                        else:
                            nc.vector.tensor_copy(dst, ps_wh[:])
                    qt = sbA.tile([128, (ACHUNK // 128) * 8], F32, tag="qt")
                    nc.scalar.activation(qt[:], ps_q[:], AF.Exp)
                    nc.sync.dma_start(
                        tw[c0 : c0 + ACHUNK, 0:256].rearrange(
                            "(k p) e -> p k e", p=128
                        ),
                        wt[:].rearrange("p (k e) -> p k e", e=256),
                    )
                    nc.sync.dma_start(
                        tw[c0 : c0 + ACHUNK, 256:272]
                        .bitcast(F32)
                        .rearrange("(k p) e -> p k e", p=128),
                        qt[:].rearrange("p (k e) -> p k e", e=8),
                    )
                # p1/p1' for own (permuted) nodes
                for w in range(NW):
                    ps_p = psA.tile([128, 8], F32, tag="ps_p")
                    nc.tensor.matmul(
                        ps_p[:],
                        xo_all[:, 128 * w : 128 * (w + 1)],
                        wextsb[:, 264:272],
                        start=True,
                        stop=True,
                    )
                    nc.scalar.activation(
                        p1sb[:, 8 * w : 8 * (w + 1)], ps_p[:], AF.Exp
                    )

        # ======== phase B: layer-1 edge processing ========
        with (
            tc.tile_pool(name="psB", bufs=4, space="PSUM") as psB,
            tc.tile_pool(name="sbB", bufs=2) as sbB,
        ):
            for _rb in range(reps.get("B", 1) if "B" in phases else 0):
                for w in range(NW):
                    G = gs[w]
                    wn = min(128, NPC - 128 * w)
                    g1 = sbB.tile([128, (GMAX + 1) * RW1], BF16, tag="g1")
                    gx = g1[:].rearrange("p (g c) -> p g c", c=RW1)
                    g3 = gx[:, :G, :]
                    for s0, sq, ioff in cfg.calls[w]:
                        nc.gpsimd.dma_gather(
                            gx[:, s0 : s0 + sq + 1, :],
                            tw[RB:, :],
                            i16_1[:, ioff : ioff + sq * 8 + 1],
                            sq * 128 + 16,
                            sq * 128 + 16,
                            RW1,
                            queue_num=q_of(w),
                        )
                    # ex = max(p1*q2, p1'*q2')  [128, G, 4] (h-minor)
                    q2p = g3[:, :, 256:272].bitcast(F32)  # [128, G, 8] f32
                    tab = sbB.tile([128, GMAX * 8], F32, tag="tab")
                    nc.vector.tensor_tensor(
                        tab[:, : G * 8].rearrange("p (g c) -> p g c", c=8),
                        q2p,
                        p1sb[:, 8 * w : 8 * (w + 1)]
                        .unsqueeze(1)
                        .to_broadcast([128, G, 8]),
                        op=ALU.mult,
                    )
                    t3 = tab[:, : G * 8].rearrange("p (g c) -> p g c", c=8)
                    ex = sbB.tile([128, GMAX * 4], BF16, tag="ex")
                    ex3 = ex[:, : G * 4].rearrange("p (g h) -> p g h", h=4)
                    nc.vector.tensor_tensor(
                        ex3, t3[:, :, 0:4], t3[:, :, 4:8], op=ALU.max
                    )
                    # scale Wh by ex IN PLACE in the gathered tile
                    nc.vector.tensor_tensor(
                        g3[:, :, 0:256].rearrange("p g (o h) -> p g o h", h=4),
                        g3[:, :, 0:256].rearrange("p g (o h) -> p g o h", h=4),
                        ex3.unsqueeze(2).to_broadcast([128, G, 64, 4]),
                        op=ALU.mult,
                    )
                    # numerator over groups (identity stationary)
                    ps_u = psB.tile([128, 256], F32, tag="ps_u")
                    for g in range(G):
                        nc.tensor.matmul(
                            ps_u[:],
                            identb[:],
                            g1[:, RW1 * g : RW1 * g + 256],
                            start=(g == 0),
                            stop=(g == G - 1),
                        )
                    # denominator: free-dim reduce of ex over groups
                    den = sbB.tile([128, 4], F32, tag="den")
                    nc.vector.tensor_reduce(
                        den[:].unsqueeze(2),
                        ex[:, : G * 4].rearrange("p (g h) -> p h g", h=4),
                        mybir.AxisListType.X,
                        ALU.add,
                    )
                    nc.vector.tensor_scalar_add(den[:], den[:], 1e-30)
                    nc.vector.reciprocal(den[:], den[:])
                    hp = sbB.tile([128, 256], BF16, tag="hp")
                    nc.vector.tensor_tensor(
                        hp[:].rearrange("p (o h) -> p o h", h=4),
                        ps_u[:, 0:256].rearrange("p (o h) -> p o h", h=4),
                        den[:].unsqueeze(1).to_broadcast([128, 64, 4]),
                        op=ALU.mult,
                    )
                    _elu_bf(
                        nc, sbB, hp[:], 256, hcat[:, 256 * w : 256 * (w + 1)], "e1"
                    )

        # ======== phase C: build own T2M shard ========
        with (
            tc.tile_pool(name="psC", bufs=4, space="PSUM") as psC,
            tc.tile_pool(name="sbC", bufs=3) as sbC,
        ):
            for _rc in range(reps.get("C", 1) if "C" in phases else 0):
                for w in range(NW):
                    wn = min(128, NPC - 128 * w)
                    ps2 = psC.tile([128, 68], F32, tag="ps2")
                    for c in range(2):
                        ps_t = psC.tile([128, 128], BF16, tag="ps_t")
                        nc.tensor.transpose(
                            ps_t[:],
                            hcat[:, 256 * w + 128 * c : 256 * w + 128 * (c + 1)],
                            identb[:],
                        )
                        hT = sbC.tile([128, 128], BF16, tag="hT")
                        nc.scalar.copy(hT[:], ps_t[:])
                        nc.tensor.matmul(
                            ps2[:],
                            hT[:],
                            w2sb[:, 68 * c : 68 * (c + 1)],
                            start=(c == 0),
                            stop=(c == 1),
                        )
                    row = sbC.tile([128, 68], BF16, tag="row")
                    nc.scalar.copy(row[:, 0:64], ps2[:, 0:64])
                    nc.scalar.activation(
                        row[:, 64:68].bitcast(F32), ps2[:, 64:66], AF.Exp
                    )
                    nc.scalar.activation(
                        p1osb[:, 2 * w : 2 * (w + 1)], ps2[:, 66:68], AF.Exp
                    )
                    nc.sync.dma_start(
                        t2msh[128 * w : 128 * w + wn, 0:68], row[:wn, :]
                    )

        # ======== phase D: allgather T2M ========
        if "D" not in phases:
            pass
        elif sim_collective:
            nc.sync.dma_start(t2m[0:NPC, :], t2msh[:])
        elif CORES > 1:
            nc.gpsimd.collective_compute(
                "AllGather",
                ALU.bypass,
                replica_groups=[list(range(CORES))],
                ins=[t2msh[:]],
                outs=[t2m[0:N, :]],
            )
        else:
            nc.sync.dma_start(t2m[0:N, :], t2msh[:])

        # ======== phase E: layer-2 edge processing ========
        with (
            tc.tile_pool(name="psE", bufs=4, space="PSUM") as psE,
            tc.tile_pool(name="sbE", bufs=2) as sbE,
        ):
            for _re in range(reps.get("E", 1) if "E" in phases else 0):
                for w in range(NW):
                    G = gs[w]
                    wn = min(128, NPC - 128 * w)
                    g2 = sbE.tile([128, (GMAX + 1) * RW2], BF16, tag="g2")
                    gx = g2[:].rearrange("p (g c) -> p g c", c=RW2)
                    g3 = gx[:, :G, :]
                    for s0, sq, ioff in cfg.calls[w]:
                        nc.gpsimd.dma_gather(
                            gx[:, s0 : s0 + sq + 1, :],
                            t2m[RB:, :],
                            i16_2[:, ioff : ioff + sq * 8 + 1],
                            sq * 128 + 16,
                            sq * 128 + 16,
                            RW2,
                            queue_num=q_of(w),
                        )
                    q2p = g3[:, :, 64:68].bitcast(F32)  # [128, G, 2]
                    tab = sbE.tile([128, GMAX * 2], F32, tag="tab2")
                    t3 = tab[:, : G * 2].rearrange("p (g c) -> p g c", c=2)
                    nc.vector.tensor_tensor(
                        t3,
                        q2p,
                        p1osb[:, 2 * w : 2 * (w + 1)]
                        .unsqueeze(1)
                        .to_broadcast([128, G, 2]),
                        op=ALU.mult,
                    )
                    ex = sbE.tile([128, GMAX], BF16, tag="ex2")
                    nc.vector.tensor_tensor(
                        ex[:, :G].unsqueeze(2),
                        t3[:, :, 0:1],
                        t3[:, :, 1:2],
                        op=ALU.max,
                    )
                    nc.vector.tensor_tensor(
                        g3[:, :, 0:64],
                        g3[:, :, 0:64],
                        ex[:, :G].unsqueeze(2).to_broadcast([128, G, 64]),
                        op=ALU.mult,
                    )
                    ps_u = psE.tile([128, 64], F32, tag="ps_u2")
                    for g in range(G):
                        nc.tensor.matmul(
                            ps_u[:],
                            identb[:],
                            g2[:, RW2 * g : RW2 * g + 64],
                            start=(g == 0),
                            stop=(g == G - 1),
                        )
                    den = sbE.tile([128, 1], F32, tag="den2")
                    nc.vector.tensor_reduce(
                        den[:].unsqueeze(2),
                        ex[:, :G].unsqueeze(1),
                        mybir.AxisListType.X,
                        ALU.add,
                    )
                    nc.vector.tensor_scalar_add(den[:], den[:], 1e-30)
                    nc.vector.reciprocal(den[:], den[:])
                    op_t = sbE.tile([128, OUT], F32, tag="op_t")
                    nc.vector.tensor_tensor(
                        op_t[:],
                        ps_u[:, 0:64],
                        den[:].to_broadcast([128, 64]),
                        op=ALU.mult,
                    )
                    _elu_bf(
                        nc, sbE, op_t[:], OUT,
                        out_all[:, OUT * w : OUT * (w + 1)], "e2", dt=F32,
                    )
                # batched output write (full windows, then the ragged tail)
                nfull = NPC // 128  # 48
                nc.sync.dma_start(
                    out_ext[0 : 128 * nfull, :].rearrange(
                        "(k p) e -> p k e", p=128
                    ),
                    out_all[:, : nfull * OUT].rearrange(
                        "p (k e) -> p k e", e=OUT
                    ),
                )
                nc.sync.dma_start(
                    out_ext[128 * nfull : NPC, :],
                    out_all[: NPC - 128 * nfull, nfull * OUT :],
                )

    nc.compile()
    return nc


# ---------------------------------------------------------------------------
# Host-side preparation and execution
# ---------------------------------------------------------------------------


def _perms_and_schedule(edges):
    src = np.asarray(edges[0], dtype=np.int64)
    deg = np.bincount(src, minlength=N)
    perms, ranks = [], []
    gw = np.zeros((CORES, NW), dtype=np.int64)
    last = np.zeros((CORES, NW), dtype=np.int64)
    for k in range(CORES):
        d = deg[k * NPC : (k + 1) * NPC]
        perm = np.argsort(-d, kind="stable")
        rank = np.empty(NPC, dtype=np.int64)
        rank[perm] = np.arange(NPC)
        perms.append(perm)
        ranks.append(rank)
        ds = np.pad(d[perm], (0, NW * 128 - NPC)).reshape(NW, 128)
        gw[k] = ds.max(axis=1)
        last[k] = ds[:, 127]
    g = gw.max(axis=0)
    g = g + (last.max(axis=0) == g)  # force last linear slot to be a pad
    g = np.maximum(g, 1)
    return perms, ranks, Cfg(g)


def make_cfg(edges):
    return _perms_and_schedule(edges)[2]


def _pack16(vals):
    """[G*128] linear slot values -> [128, G*8] int16 (16-wrap, 8 replicas)."""
    g8 = len(vals) // 16
    w = vals.reshape(g8, 16).T  # [16, G*8]
    return np.tile(w, (8, 1)).astype(np.int16)


def prepare_inputs(cfg: Cfg, x, edges, W_heads, a_heads, W_out, a_out):
    import ml_dtypes

    bf16 = ml_dtypes.bfloat16
    src = np.asarray(edges[0], dtype=np.int64)
    dst = np.asarray(edges[1], dtype=np.int64)
    x = np.asarray(x, np.float32)
    Wh = np.asarray(W_heads, np.float32)
    ah = np.asarray(a_heads, np.float32)
    Wo = np.asarray(W_out, np.float32)
    ao = np.asarray(a_out, np.float32)

    perms, ranks, _ = _perms_and_schedule(edges)

    # wext: [Wh (o,h)-order 256 | c2 4 | .2*c2 | c1 4 | .2*c1]
    wext = np.zeros((F_IN, 272), np.float32)
    for h in range(HEADS):
        wext[:, np.arange(HID) * 4 + h] = Wh[h]  # col o*4+h = Wh[h][:, o]
    c1 = np.stack([Wh[h] @ ah[h, :HID] for h in range(HEADS)], axis=1)
    c2 = np.stack([Wh[h] @ ah[h, HID:] for h in range(HEADS)], axis=1)
    wext[:, 256:260] = c2
    wext[:, 260:264] = ALPHA * c2
    wext[:, 264:268] = c1
    wext[:, 268:272] = ALPHA * c1

    # w2ext rows are hcat features in (o,h) order: row f=(o*4+h) = Wo[h*64+o]
    f = np.arange(256)
    Wop = Wo[(f % 4) * HID + (f // 4)]
    w2 = np.zeros((256, 68), np.float32)
    w2[:, 0:64] = Wop
    w2[:, 64] = Wop @ ao[OUT:]
    w2[:, 65] = ALPHA * (Wop @ ao[OUT:])
    w2[:, 66] = Wop @ ao[:OUT]
    w2[:, 67] = ALPHA * (Wop @ ao[:OUT])

    xT = np.zeros((F_IN, N2), np.float32)
    xT[:, :N] = x.T
    xT = xT.astype(bf16)

    # global layer-2 row of node v: owner*NPC + rank
    row2 = np.empty(N, dtype=np.int64)
    for k in range(CORES):
        row2[k * NPC : (k + 1) * NPC] = k * NPC + ranks[k]

    common = dict(
        wext=wext.astype(bf16),
        w2ext=np.ascontiguousarray(w2.reshape(2, 128, 68).astype(bf16)),
    )

    in_maps = []
    for k in range(CORES):
        own = (src >= k * NPC) & (src < (k + 1) * NPC)
        es = ranks[k][src[own] - k * NPC]  # rank 0..NPC-1
        ed = dst[own]
        order = np.argsort(es, kind="stable")
        es, ed = es[order], ed[order]
        counts = np.bincount(es, minlength=NW * 128)
        starts = np.concatenate([[0], np.cumsum(counts)])[:-1]
        g = np.arange(len(es)) - starts[es]
        w = es // 128
        p = es % 128
        pos = (cfg.goff[w] + g) * 128 + p
        flat1 = np.full(cfg.sg * 128, SENT1 - RB, dtype=np.int64)
        flat1[pos] = ed - RB
        flat2 = np.full(cfg.sg * 128, SENT2 - RB, dtype=np.int64)
        flat2[pos] = row2[ed] - RB
        def pack_calls(flat, sent):
            parts = []
            for w in range(NW):
                base = 128 * cfg.goff[w]
                for s0, sq, _ in cfg.calls[w]:
                    v = flat[base + 128 * s0 : base + 128 * (s0 + sq)]
                    v = np.concatenate([v, np.full(16, sent, np.int64)])
                    parts.append(_pack16(v))
            return np.concatenate(parts, axis=1)

        i1 = pack_calls(flat1, SENT1 - RB)
        i2 = pack_calls(flat2, SENT2 - RB)
        xo = np.zeros((F_IN, NW * 128), np.float32)
        xo[:, :NPC] = x.T[:, k * NPC + perms[k]]
        in_maps.append(
            dict(
                common,
                xT=xT,
                xTown=xo.astype(bf16),
                idx1=np.ascontiguousarray(i1),
                idx2=np.ascontiguousarray(i2),
            )
        )
    return in_maps, perms


_NC_CACHE = {}


def get_nc(cfg: Cfg):
    key = cfg.gs
    if key not in _NC_CACHE:
        _NC_CACHE[key] = build_nc(cfg)
    return _NC_CACHE[key]


def run(inputs, trace=False, **spmd_kwargs):
    from concourse.bass_utils import run_bass_kernel_spmd

    edges = np.asarray(inputs["edges"])
    cfg = make_cfg(edges)
    nc = get_nc(cfg)
    in_maps, perms = prepare_inputs(
        cfg,
        inputs["x"],
        edges,
        inputs["W_heads"],
        inputs["a_heads"],
        inputs["W_out"],
        inputs["a_out"],
    )
    res = run_bass_kernel_spmd(
        nc, in_maps, core_ids=list(range(CORES)), trace=trace, **spmd_kwargs
    )
    out = np.zeros((N, OUT), np.float32)
    for k in range(CORES):
        out[k * NPC + perms[k]] = res.results[k]["out"]
    return out, res


def kernel(**inputs):
    return run(inputs)[0]


# revision 7
# speedup vs baseline: 1322.0060x; 1.3421x over previous
"""Trainium2 Bass kernel v2 for 2-layer multi-head GAT (nn_GAT_38551626449703).

Design (8 NeuronCores, SPMD, one shared program):
  - Core k owns nodes [k*NPC, (k+1)*NPC).  Within a core, owned nodes are
    PERMUTED by out-degree (descending) and windowed 128 at a time; the
    shared per-window group count G_w = max over cores of the window's max
    degree (3-4% slot padding).  Host un-permutes the output rows.
  - Edge slots: window w, partition p, group g holds the g-th edge of the
    window's p-th node, so PARTITION p == SRC NODE p.  No one-hot matrix,
    no src gather: per-node src factors broadcast along the free dim.
  - exp/LeakyReLU factorization (exact, since exp is monotonic):
        ex = exp(lrelu(s1+s2)) = max(p1[src]*q2[dst], p1'[src]*q2'[dst])
    with p1=exp(s1), p1'=exp(.2*s1) per src node (SBUF-resident) and
    q2=exp(s2), q2'=exp(.2*s2) stored per dst node in the gather tables.
  - Gather tables (row dtype bf16; the f32 q-pairs live in the row pad):
      TW  [N2+1, 384]: [Wh (o,h)-order 256 | q2 f32x4 | q2' f32x4 | pad]
                        768B rows; row N2 is an all-zero sentinel.
      T2M [N+1, 128]:  [Wh2 64 | q2o f32 | q2o' f32 | pad] 256B rows;
                        rows are permuted-block (owner*NPC + rank);
                        row N is the sentinel.
    One dma_gather per window per layer: int16 indices with table base at
    row RB=32768 cover all rows via SIGNED offsets (verified on HW).
    Pad slots gather the sentinel row (q2=0 -> ex==0 exactly).
  - Accumulation: per group g one matmul with a STATIONARY bf16 identity,
    rhs = [Wh*ex | ex] -> PSUM f32 [128, 260] accumulates numerator+denom.
  - Wh columns are stored in (o,h) order so the ex broadcast multiplies
    keep the last AP dim packed (DVE 2x mode).
  - Between layers only the compact T2M shard (1.6 MB bf16) is AllGathered.
"""

import os
import sys

import numpy as np

sys.path.insert(0, "/opt/trn_rl_repo")

import concourse.bacc as bacc  # noqa: E402
import concourse.tile as tile  # noqa: E402
from concourse import mybir  # noqa: E402
from concourse.masks import make_identity  # noqa: E402

F32 = mybir.dt.float32
BF16 = mybir.dt.bfloat16
I16 = mybir.dt.int16
AF = mybir.ActivationFunctionType
ALU = mybir.AluOpType

# Problem constants
N = 50000
E = 800000
F_IN = 128
HID = 64
HEADS = 4
OUT = 64
ALPHA = 0.2
CORES = 8
NPC = N // CORES  # 6250
NW = (NPC + 127) // 128  # 49

RB = 32768  # gather table base row (int16 signed-offset trick)
GSPLIT = 7  # max groups per dma_gather call (7*128+16=912 <= 1024-desc ring)
RW1 = 384  # TW row, bf16 elems (768B)
RW2 = 128  # T2M row, bf16 elems (256B)
ACHUNK = 2048  # phase-A nodes per chunk
N2 = ((N + ACHUNK - 1) // ACHUNK) * ACHUNK  # 51200; rows N..N2-1 zero-x junk
NT1 = N2 // ACHUNK  # 25
SENT1 = N2  # TW sentinel row
SENT2 = N  # T2M sentinel row

NSWQ = int(os.environ.get("GAT_NSWQ", "4"))  # SWDGE queues

# Tile assigns the 8 DMASW completion-sem lanes round-robin over Pool-engine
# DMAs regardless of SWDGE queue, but a lane must stay on ONE queue (ucode
# constraint).  With NSWQ > 1 we partition the lanes per queue.
if NSWQ > 1:
    import concourse.bass_isa as _bass_isa
    import concourse.tile_sem_assignment as _tsa

    _orig_assign_tick = _tsa.TileClockTick._assign_tick

    def _lane_partitioned_assign_tick(self, inst):
        if (
            isinstance(inst, _tsa.DMAInst)
            and inst.engine == mybir.EngineType.Pool
            and not isinstance(inst, _bass_isa.UserSyncedRemoteDMADescs)
        ):
            qn = getattr(inst, "queue_num", 0) or 0
            per = getattr(self, "_q_lane_ctr", None)
            if per is None:
                per = self._q_lane_ctr = {}
            lanes = 8 // NSWQ
            c = per.get(qn, 0)
            per[qn] = c + 1
            self.next_sw_dma_idx = qn * lanes + (c % lanes)
        return _orig_assign_tick(self, inst)

    if _tsa.TileClockTick._assign_tick is not _lane_partitioned_assign_tick:
        _tsa.TileClockTick._assign_tick = _lane_partitioned_assign_tick


class Cfg:
    def __init__(self, gs):
        self.gs = tuple(int(g) for g in gs)  # per-window group counts
        assert len(self.gs) == NW
        self.sg = sum(self.gs)
        self.goff = np.concatenate([[0], np.cumsum(self.gs)]).astype(int)
        self.gmax = max(self.gs)
        # per-window gather calls: (s0, sq, idx col offset).  Each call's
        # index list gets 16 appended sentinel slots (idx >= 0) because the
        # ucode SKIPS a trailing run of negative indices (HW-probed); the
        # junk slots land in group s0+sq, partitions 0..15.
        self.calls = []
        off = 0
        for g in self.gs:
            wcalls = []
            for s0 in range(0, g, GSPLIT):
                sq = min(GSPLIT, g - s0)
                wcalls.append((s0, sq, off))
                off += sq * 8 + 1
            self.calls.append(wcalls)
        self.icols = off


def _elu_bf(nc, pool, x_ap, cols, out_ap, tag, dt=BF16):
    """out = elu(x) = relu(x) + exp(x - relu(x)) - 1.  relu/exp on ACT."""
    rl = pool.tile([128, cols], dt, tag=f"{tag}_rl")
    nc.scalar.activation(rl[:], x_ap, AF.Relu)
    t = pool.tile([128, cols], dt, tag=f"{tag}_t")
    nc.vector.tensor_tensor(t[:], x_ap, rl[:], op=ALU.subtract)
    nc.scalar.activation(t[:], t[:], AF.Exp)
    # out = (rl + (-1)) + t
    nc.vector.scalar_tensor_tensor(
        out=out_ap, in0=rl[:], scalar=-1.0, in1=t[:], op0=ALU.add, op1=ALU.add
    )


def build_nc(cfg: Cfg, reps=None, sim_collective: bool = False):
    reps = reps or {}
    phases = os.environ.get("GAT_PHASES", "ABCDE")
    gs, goff = cfg.gs, cfg.goff
    SG8 = 8 * cfg.sg
    GMAX = cfg.gmax

    # One dma_gather call's descriptors must fit the SWDGE ring (default
    # dynamic_dma_scratch 16384B = 1024 descriptors); GSPLIT=7 groups
    # (896+16 descriptors) is HW-validated under that limit.
    nc = bacc.Bacc(
        "TRN2",
        target_bir_lowering=False,
        debug=False,
        num_swdge_queues=NSWQ,
    )

    # ---- external I/O ----
    xT_ext = nc.dram_tensor("xT", [F_IN, N2], BF16, kind="ExternalInput")
    xTo_ext = nc.dram_tensor("xTown", [F_IN, NW * 128], BF16, kind="ExternalInput")
    wext_ext = nc.dram_tensor("wext", [F_IN, 272], BF16, kind="ExternalInput")
    w2ext_ext = nc.dram_tensor("w2ext", [2, 128, 68], BF16, kind="ExternalInput")
    idx1_ext = nc.dram_tensor("idx1", [128, cfg.icols], I16, kind="ExternalInput")
    idx2_ext = nc.dram_tensor("idx2", [128, cfg.icols], I16, kind="ExternalInput")
    out_ext = nc.dram_tensor("out", [NPC, OUT], F32, kind="ExternalOutput")

    # ---- internal DRAM ----
    tw = nc.dram_tensor("TW", [N2 + 1, RW1], BF16)
    t2msh = nc.dram_tensor("T2Msh", [NPC, RW2], BF16)
    if CORES > 1 and not sim_collective:
        t2m = nc.dram_tensor("T2M", [N + 1, RW2], BF16, addr_space="Shared")
    else:
        t2m = nc.dram_tensor("T2M", [N + 1, RW2], BF16)

    def q_of(w):
        return w % NSWQ

    with tile.TileContext(nc) as tc, tc.tile_pool(name="const", bufs=1) as cpool:
        # ======== persistent SBUF ========
        ident = cpool.tile([128, 128], F32)
        make_identity(nc, ident[:])
        identb = cpool.tile([128, 128], BF16)
        nc.vector.tensor_copy(identb[:], ident[:])
        wextsb = cpool.tile([F_IN, 272], BF16)
        nc.sync.dma_start(wextsb[:], wext_ext[:])
        w2sb = cpool.tile([128, 2 * 68], BF16)
        nc.sync.dma_start(w2sb[:, 0:68], w2ext_ext[0])
        nc.sync.dma_start(w2sb[:, 68:136], w2ext_ext[1])
        i16_1 = cpool.tile([128, cfg.icols], I16)
        nc.sync.dma_start(i16_1[:], idx1_ext[:])
        i16_2 = cpool.tile([128, cfg.icols], I16)
        nc.sync.dma_start(i16_2[:], idx2_ext[:])
        p1sb = cpool.tile([128, NW * 8], F32)
        p1osb = cpool.tile([128, NW * 2], F32)
        hcat = cpool.tile([128, NW * 256], BF16)
        out_all = cpool.tile([128, NW * OUT], F32)
        xo_all = cpool.tile([128, NW * 128], BF16)
        nc.sync.dma_start(xo_all[:], xTo_ext[:])

        # sentinel rows (all zeros)
        z1 = cpool.tile([1, RW1], BF16)
        nc.vector.memset(z1[:], 0.0)
        nc.sync.dma_start(tw[SENT1 : SENT1 + 1, :], z1[:])
        z2 = cpool.tile([1, RW2], BF16)
        nc.vector.memset(z2[:], 0.0)
        nc.sync.dma_start(t2m[SENT2 : SENT2 + 1, :], z2[:])

        # ======== phase A: build TW (all nodes) + p1/p1' (own nodes) ======
        with (
            tc.tile_pool(name="psA", bufs=2, space="PSUM") as psA,
            tc.tile_pool(name="sbA", bufs=3) as sbA,
        ):
            for _ra in range(reps.get("A", 1) if "A" in phases else 0):
                for t in range(NT1):
                    c0 = ACHUNK * t
                    xt = sbA.tile([128, ACHUNK], BF16, tag="xt")
                    nc.sync.dma_start(xt[:], xT_ext[:, c0 : c0 + ACHUNK])
                    wt = sbA.tile([128, (ACHUNK // 128) * 256], BF16, tag="wt")
                    ps_q = psA.tile([128, (ACHUNK // 128) * 8], F32, tag="ps_q")
                    for par in range(ACHUNK // 256):
                        ps_wh = psA.tile([128, 512], F32, tag="ps_wh")
                        for h in range(2):
                            q = 2 * par + h
                            nc.tensor.matmul(
                                ps_wh[:, 256 * h : 256 * (h + 1)],
                                xt[:, 128 * q : 128 * (q + 1)],
                                wextsb[:, 0:256],
                                start=True,
                                stop=True,
                            )
                            nc.tensor.matmul(
                                ps_q[:, 8 * q : 8 * (q + 1)],
                                xt[:, 128 * q : 128 * (q + 1)],
                                wextsb[:, 256:264],
                                start=True,
                                stop=True,
                            )
                        # convert Wh pair to bf16 on ACT (DVE is the
                        # busier engine overall; ACT idles in phase A)
                        dst = wt[:, 512 * par : 512 * (par + 1)]
                        nc.scalar.copy(dst, ps_wh[:])
                    qt = sbA.tile([128, (ACHUNK // 128) * 8], F32, tag="qt")
                    nc.scalar.activation(qt[:], ps_q[:], AF.Exp)
                    nc.sync.dma_start(
                        tw[c0 : c0 + ACHUNK, 0:256].rearrange(
                            "(k p) e -> p k e", p=128
                        ),
                        wt[:].rearrange("p (k e) -> p k e", e=256),
                    )
                    nc.sync.dma_start(
                        tw[c0 : c0 + ACHUNK, 256:272]
                        .bitcast(F32)
                        .rearrange("(k p) e -> p k e", p=128),
                        qt[:].rearrange("p (k e) -> p k e", e=8),
                    )
                # p1/p1' for own (permuted) nodes
                for w in range(NW):
                    ps_p = psA.tile([128, 8], F32, tag="ps_p")
                    nc.tensor.matmul(
                        ps_p[:],
                        xo_all[:, 128 * w : 128 * (w + 1)],
                        wextsb[:, 264:272],
                        start=True,
                        stop=True,
                    )
                    nc.scalar.activation(
                        p1sb[:, 8 * w : 8 * (w + 1)], ps_p[:], AF.Exp
                    )

        # ======== phase B: layer-1 edge processing ========
        with (
            tc.tile_pool(name="psB", bufs=2, space="PSUM") as psB,
            tc.tile_pool(name="sbB", bufs=2) as sbB,
        ):
            for _rb in range(reps.get("B", 1) if "B" in phases else 0):
                for w in range(NW):
                    G = gs[w]
                    wn = min(128, NPC - 128 * w)
                    # each call gets a DISJOINT span [s0+i, s0+i+sq+1) in the
                    # tile: its 16 sentinel-junk slots land in a dead gap
                    # group, so calls neither serialize nor corrupt each other
                    ncall = len(cfg.calls[w])
                    g1 = sbB.tile([128, (GMAX + 6) * RW1], BF16, tag="g1")
                    gx = g1[:].rearrange("p (g c) -> p g c", c=RW1)
                    tab = sbB.tile([128, GMAX * 8], F32, tag="tab")
                    ex = sbB.tile([128, GMAX * 4], BF16, tag="ex")
                    ex3 = ex[:, : G * 4].rearrange("p (g h) -> p g h", h=4)
                    for i, (s0, sq, ioff) in enumerate(cfg.calls[w]):
                        nc.gpsimd.dma_gather(
                            gx[:, s0 + i : s0 + i + sq + 1, :],
                            tw[RB:, :],
                            i16_1[:, ioff : ioff + sq * 8 + 1],
                            sq * 128 + 16,
                            sq * 128 + 16,
                            RW1,
                            queue_num=(w + i) % NSWQ,
                        )
                    for i, (s0, sq, ioff) in enumerate(cfg.calls[w]):
                        sp = gx[:, s0 + i : s0 + i + sq, :]
                        nc.vector.tensor_tensor(
                            tab[:, 8 * s0 : 8 * (s0 + sq)].rearrange(
                                "p (g c) -> p g c", c=8
                            ),
                            sp[:, :, 256:272].bitcast(F32),
                            p1sb[:, 8 * w : 8 * (w + 1)]
                            .unsqueeze(1)
                            .to_broadcast([128, sq, 8]),
                            op=ALU.mult,
                        )
                    t3 = tab[:, : G * 8].rearrange("p (g c) -> p g c", c=8)
                    nc.vector.tensor_tensor(
                        ex3, t3[:, :, 0:4], t3[:, :, 4:8], op=ALU.max
                    )
                    # scale Wh by ex IN PLACE per call span
                    for i, (s0, sq, ioff) in enumerate(cfg.calls[w]):
                        sp = gx[:, s0 + i : s0 + i + sq, :]
                        nc.vector.tensor_tensor(
                            sp[:, :, 0:256].rearrange("p g (o h) -> p g o h", h=4),
                            sp[:, :, 0:256].rearrange("p g (o h) -> p g o h", h=4),
                            ex3[:, s0 : s0 + sq, :]
                            .unsqueeze(2)
                            .to_broadcast([128, sq, 64, 4]),
                            op=ALU.mult,
                        )
                    # numerator over groups (identity stationary)
                    ps_u = psB.tile([128, 256], F32, tag="ps_u")
                    for g in range(G):
                        tg = g + g // GSPLIT  # tile group with gap shift
                        nc.tensor.matmul(
                            ps_u[:],
                            identb[:],
                            g1[:, RW1 * tg : RW1 * tg + 256],
                            start=(g == 0),
                            stop=(g == G - 1),
                        )
                    # denominator: free-dim reduce of ex over groups
                    den = sbB.tile([128, 4], F32, tag="den")
                    nc.vector.tensor_reduce(
                        den[:].unsqueeze(2),
                        ex[:, : G * 4].rearrange("p (g h) -> p h g", h=4),
                        mybir.AxisListType.X,
                        ALU.add,
                    )
                    nc.vector.tensor_scalar_add(den[:], den[:], 1e-30)
                    nc.vector.reciprocal(den[:], den[:])
                    hp = sbB.tile([128, 256], BF16, tag="hp")
                    nc.vector.tensor_tensor(
                        hp[:].rearrange("p (o h) -> p o h", h=4),
                        ps_u[:, 0:256].rearrange("p (o h) -> p o h", h=4),
                        den[:].unsqueeze(1).to_broadcast([128, 64, 4]),
                        op=ALU.mult,
                    )
                    _elu_bf(
                        nc, sbB, hp[:], 256, hcat[:, 256 * w : 256 * (w + 1)], "e1"
                    )
                    # ---- fused phase C: this window's T2M shard rows ----
                    ps2 = psB.tile([128, 68], F32, tag="ps2")
                    for c in range(2):
                        ps_t = psB.tile([128, 128], BF16, tag="ps_t")
                        nc.tensor.transpose(
                            ps_t[:],
                            hcat[:, 256 * w + 128 * c : 256 * w + 128 * (c + 1)],
                            identb[:],
                        )
                        hT = sbB.tile([128, 128], BF16, tag="hT")
                        nc.scalar.copy(hT[:], ps_t[:])
                        nc.tensor.matmul(
                            ps2[:],
                            hT[:],
                            w2sb[:, 68 * c : 68 * (c + 1)],
                            start=(c == 0),
                            stop=(c == 1),
                        )
                    row = sbB.tile([128, 68], BF16, tag="row")
                    nc.scalar.copy(row[:, 0:64], ps2[:, 0:64])
                    nc.scalar.activation(
                        row[:, 64:68].bitcast(F32), ps2[:, 64:66], AF.Exp
                    )
                    nc.scalar.activation(
                        p1osb[:, 2 * w : 2 * (w + 1)], ps2[:, 66:68], AF.Exp
                    )
                    nc.sync.dma_start(
                        t2msh[128 * w : 128 * w + wn, 0:68], row[:wn, :]
                    )

        # ======== phase D: allgather T2M ========
        if "D" not in phases:
            pass
        elif sim_collective:
            nc.sync.dma_start(t2m[0:NPC, :], t2msh[:])
        elif CORES > 1:
            nc.gpsimd.collective_compute(
                "AllGather",
                ALU.bypass,
                replica_groups=[list(range(CORES))],
                ins=[t2msh[:]],
                outs=[t2m[0:N, :]],
            )
        else:
            nc.sync.dma_start(t2m[0:N, :], t2msh[:])

        # ======== phase E: layer-2 edge processing ========
        with (
            tc.tile_pool(name="psE", bufs=4, space="PSUM") as psE,
            tc.tile_pool(name="sbE", bufs=2) as sbE,
        ):
            for _re in range(reps.get("E", 1) if "E" in phases else 0):
                for w in range(NW):
                    G = gs[w]
                    wn = min(128, NPC - 128 * w)
                    ncall = len(cfg.calls[w])
                    g2 = sbE.tile([128, (GMAX + 6) * RW2], BF16, tag="g2")
                    gx = g2[:].rearrange("p (g c) -> p g c", c=RW2)
                    tab = sbE.tile([128, GMAX * 2], F32, tag="tab2")
                    ex = sbE.tile([128, GMAX], BF16, tag="ex2")
                    for i, (s0, sq, ioff) in enumerate(cfg.calls[w]):
                        nc.gpsimd.dma_gather(
                            gx[:, s0 + i : s0 + i + sq + 1, :],
                            t2m[RB:, :],
                            i16_2[:, ioff : ioff + sq * 8 + 1],
                            sq * 128 + 16,
                            sq * 128 + 16,
                            RW2,
                            queue_num=(w + i) % NSWQ,
                        )
                    for i, (s0, sq, ioff) in enumerate(cfg.calls[w]):
                        sp = gx[:, s0 + i : s0 + i + sq, :]
                        nc.vector.tensor_tensor(
                            tab[:, 2 * s0 : 2 * (s0 + sq)].rearrange(
                                "p (g c) -> p g c", c=2
                            ),
                            sp[:, :, 64:68].bitcast(F32),
                            p1osb[:, 2 * w : 2 * (w + 1)]
                            .unsqueeze(1)
                            .to_broadcast([128, sq, 2]),
                            op=ALU.mult,
                        )
                    t3 = tab[:, : G * 2].rearrange("p (g c) -> p g c", c=2)
                    nc.vector.tensor_tensor(
                        ex[:, :G].unsqueeze(2),
                        t3[:, :, 0:1],
                        t3[:, :, 1:2],
                        op=ALU.max,
                    )
                    for i, (s0, sq, ioff) in enumerate(cfg.calls[w]):
                        sp = gx[:, s0 + i : s0 + i + sq, :]
                        nc.vector.tensor_tensor(
                            sp[:, :, 0:64],
                            sp[:, :, 0:64],
                            ex[:, s0 : s0 + sq]
                            .unsqueeze(2)
                            .to_broadcast([128, sq, 64]),
                            op=ALU.mult,
                        )
                    ps_u = psE.tile([128, 64], F32, tag="ps_u2")
                    for g in range(G):
                        tg = g + g // GSPLIT
                        nc.tensor.matmul(
                            ps_u[:],
                            identb[:],
                            g2[:, RW2 * tg : RW2 * tg + 64],
                            start=(g == 0),
                            stop=(g == G - 1),
                        )
                    den = sbE.tile([128, 1], F32, tag="den2")
                    nc.vector.tensor_reduce(
                        den[:].unsqueeze(2),
                        ex[:, :G].unsqueeze(1),
                        mybir.AxisListType.X,
                        ALU.add,
                    )
                    nc.vector.tensor_scalar_add(den[:], den[:], 1e-30)
                    nc.vector.reciprocal(den[:], den[:])
                    op_t = sbE.tile([128, OUT], F32, tag="op_t")
                    nc.vector.tensor_tensor(
                        op_t[:],
                        ps_u[:, 0:64],
                        den[:].to_broadcast([128, 64]),
                        op=ALU.mult,
                    )
                    _elu_bf(
                        nc, sbE, op_t[:], OUT,
                        out_all[:, OUT * w : OUT * (w + 1)], "e2", dt=F32,
                    )
                # batched output write (full windows, then the ragged tail)
                nfull = NPC // 128  # 48
                nc.sync.dma_start(
                    out_ext[0 : 128 * nfull, :].rearrange(
                        "(k p) e -> p k e", p=128
                    ),
                    out_all[:, : nfull * OUT].rearrange(
                        "p (k e) -> p k e", e=OUT
                    ),
                )
                nc.sync.dma_start(
                    out_ext[128 * nfull : NPC, :],
                    out_all[: NPC - 128 * nfull, nfull * OUT :],
                )

    nc.compile()
    return nc


# ---------------------------------------------------------------------------
# Host-side preparation and execution
# ---------------------------------------------------------------------------


def _perms_and_schedule(edges):
    src = np.asarray(edges[0], dtype=np.int64)
    deg = np.bincount(src, minlength=N)
    perms, ranks = [], []
    gw = np.zeros((CORES, NW), dtype=np.int64)
    last = np.zeros((CORES, NW), dtype=np.int64)
    for k in range(CORES):
        d = deg[k * NPC : (k + 1) * NPC]
        perm = np.argsort(-d, kind="stable")
        rank = np.empty(NPC, dtype=np.int64)
        rank[perm] = np.arange(NPC)
        perms.append(perm)
        ranks.append(rank)
        ds = np.pad(d[perm], (0, NW * 128 - NPC)).reshape(NW, 128)
        gw[k] = ds.max(axis=1)
        last[k] = ds[:, 127]
    g = gw.max(axis=0)
    g = g + (last.max(axis=0) == g)  # force last linear slot to be a pad
    g = np.maximum(g, 1)
    return perms, ranks, Cfg(g)


def make_cfg(edges):
    return _perms_and_schedule(edges)[2]


def _pack16(vals):
    """[G*128] linear slot values -> [128, G*8] int16 (16-wrap, 8 replicas)."""
    g8 = len(vals) // 16
    w = vals.reshape(g8, 16).T  # [16, G*8]
    return np.tile(w, (8, 1)).astype(np.int16)


def prepare_inputs(cfg: Cfg, x, edges, W_heads, a_heads, W_out, a_out):
    import ml_dtypes

    bf16 = ml_dtypes.bfloat16
    src = np.asarray(edges[0], dtype=np.int64)
    dst = np.asarray(edges[1], dtype=np.int64)
    x = np.asarray(x, np.float32)
    Wh = np.asarray(W_heads, np.float32)
    ah = np.asarray(a_heads, np.float32)
    Wo = np.asarray(W_out, np.float32)
    ao = np.asarray(a_out, np.float32)

    perms, ranks, _ = _perms_and_schedule(edges)

    # wext: [Wh (o,h)-order 256 | c2 4 | .2*c2 | c1 4 | .2*c1]
    wext = np.zeros((F_IN, 272), np.float32)
    for h in range(HEADS):
        wext[:, np.arange(HID) * 4 + h] = Wh[h]  # col o*4+h = Wh[h][:, o]
    c1 = np.stack([Wh[h] @ ah[h, :HID] for h in range(HEADS)], axis=1)
    c2 = np.stack([Wh[h] @ ah[h, HID:] for h in range(HEADS)], axis=1)
    wext[:, 256:260] = c2
    wext[:, 260:264] = ALPHA * c2
    wext[:, 264:268] = c1
    wext[:, 268:272] = ALPHA * c1

    # w2ext rows are hcat features in (o,h) order: row f=(o*4+h) = Wo[h*64+o]
    f = np.arange(256)
    Wop = Wo[(f % 4) * HID + (f // 4)]
    w2 = np.zeros((256, 68), np.float32)
    w2[:, 0:64] = Wop
    w2[:, 64] = Wop @ ao[OUT:]
    w2[:, 65] = ALPHA * (Wop @ ao[OUT:])
    w2[:, 66] = Wop @ ao[:OUT]
    w2[:, 67] = ALPHA * (Wop @ ao[:OUT])

    xT = np.zeros((F_IN, N2), np.float32)
    xT[:, :N] = x.T
    xT = xT.astype(bf16)

    # global layer-2 row of node v: owner*NPC + rank
    row2 = np.empty(N, dtype=np.int64)
    for k in range(CORES):
        row2[k * NPC : (k + 1) * NPC] = k * NPC + ranks[k]

    common = dict(
        wext=wext.astype(bf16),
        w2ext=np.ascontiguousarray(w2.reshape(2, 128, 68).astype(bf16)),
    )

    in_maps = []
    for k in range(CORES):
        own = (src >= k * NPC) & (src < (k + 1) * NPC)
        es = ranks[k][src[own] - k * NPC]  # rank 0..NPC-1
        ed = dst[own]
        order = np.argsort(es, kind="stable")
        es, ed = es[order], ed[order]
        counts = np.bincount(es, minlength=NW * 128)
        starts = np.concatenate([[0], np.cumsum(counts)])[:-1]
        g = np.arange(len(es)) - starts[es]
        w = es // 128
        p = es % 128
        pos = (cfg.goff[w] + g) * 128 + p
        flat1 = np.full(cfg.sg * 128, SENT1 - RB, dtype=np.int64)
        flat1[pos] = ed - RB
        flat2 = np.full(cfg.sg * 128, SENT2 - RB, dtype=np.int64)
        flat2[pos] = row2[ed] - RB
        def pack_calls(flat, sent):
            parts = []
            for w in range(NW):
                base = 128 * cfg.goff[w]
                for s0, sq, _ in cfg.calls[w]:
                    v = flat[base + 128 * s0 : base + 128 * (s0 + sq)]
                    v = np.concatenate([v, np.full(16, sent, np.int64)])
                    parts.append(_pack16(v))
            return np.concatenate(parts, axis=1)

        i1 = pack_calls(flat1, SENT1 - RB)
        i2 = pack_calls(flat2, SENT2 - RB)
        xo = np.zeros((F_IN, NW * 128), np.float32)
        xo[:, :NPC] = x.T[:, k * NPC + perms[k]]
        in_maps.append(
            dict(
                common,
                xT=xT,
                xTown=xo.astype(bf16),
                idx1=np.ascontiguousarray(i1),
                idx2=np.ascontiguousarray(i2),
            )
        )
    return in_maps, perms


_NC_CACHE = {}


def get_nc(cfg: Cfg):
    key = cfg.gs
    if key not in _NC_CACHE:
        _NC_CACHE[key] = build_nc(cfg)
    return _NC_CACHE[key]


def run(inputs, trace=False, **spmd_kwargs):
    from concourse.bass_utils import run_bass_kernel_spmd

    edges = np.asarray(inputs["edges"])
    cfg = make_cfg(edges)
    nc = get_nc(cfg)
    in_maps, perms = prepare_inputs(
        cfg,
        inputs["x"],
        edges,
        inputs["W_heads"],
        inputs["a_heads"],
        inputs["W_out"],
        inputs["a_out"],
    )
    res = run_bass_kernel_spmd(
        nc, in_maps, core_ids=list(range(CORES)), trace=trace, **spmd_kwargs
    )
    out = np.zeros((N, OUT), np.float32)
    for k in range(CORES):
        out[k * NPC + perms[k]] = res.results[k]["out"]
    return out, res


def kernel(**inputs):
    return run(inputs)[0]
